# revision 37
# baseline (speedup 1.0000x reference)
"""Distributed GraphSAGE (3-layer, mean aggregation) on 8 Trainium2 NeuronCores.

Strategy (ClusterGCN-style node partitioning), v2:
  - Nodes are load-balance-packed into 128-slot "blocks" (degree-aware snake
    packing), 49 blocks per core -> each core owns 6272 node slots.
  - The gather table (h, bf16) is split into two parts: part1 = every core's
    blocks 0..24, part2 = blocks 25..48.  Each part is AllGathered as soon as
    its blocks finish, so collectives overlap compute, and each part stays
    below the int16 index range of dma_gather.
  - Per layer, two passes over the 49 destination blocks: pass A aggregates
    part1-sourced edges (partials parked in SBUF via the Scalar engine),
    pass B aggregates part2-sourced edges, combines, and runs the epilogue.
  - v2 changes vs v1 (which was SWDGE descriptor-generation bound):
    * Gather calls are MERGED across destination blocks (up to 8192 indices
      per dma_gather) to amortize the ~1-2us fixed SWDGE cost. Per-block
      edge caps are padded to %128 so block boundaries align with matmul
      subtiles; pad indices gather row 0 and are killed by the one-hot.
    * Layer-0 messages (x[src]) are pre-gathered on the host into the exact
      SBUF tile layout and streamed with HWDGE dma_start - no descriptor
      generation at all for layer 0.
    * One-hot builds (Vector engine is_equal) are batched over pairs of
      blocks to amortize DVE dispatch overhead.
    * The mean scaling is folded into the transpose matmul: meanT =
      mean_pre^T @ diag(deg_inv_block), replacing the Scalar-engine scale.
    * The dense epilogue is flipped: densT[K, slot] = Wl^T meanT + Wr^T hT,
      so bias+ReLU fuse into one Scalar activation (bias is per-partition)
      and h^T lands directly in the next layer's hT strip. This removes all
      bias matmuls and one transpose per block.
    * PSUM->SBUF copies moved from Vector to Scalar engine; weights / hT
      strips in bf16.
  - Output is produced transposed ([out_f, slots]); the host unshards.
"""

import sys

if "/opt/trn_rl_repo" not in sys.path:
    sys.path.insert(0, "/opt/trn_rl_repo")

import os
import numpy as np
import ml_dtypes

BF16 = ml_dtypes.bfloat16
P = 128
MAXN = int(os.environ.get("GNN_MAXN", "1024"))  # max indices per dma_gather
# single_packet mode for dma_gather: "auto" = True iff MAXN <= 1024
_SP = os.environ.get("GNN_SP", "auto")
SP = (MAXN <= 1024) if _SP == "auto" else (_SP == "1")
# SWDGE descriptor-ring carveout (bytes per partition).  Rings are per
# (engine, queue) partition rows: 16KiB/16B = 1024 descs per engine per queue
# (~15 calls deep at MAXN=1024) - the default is plenty.  (The MAXN=2048/4096
# regressions were intrinsic per-desc gen slowdown at larger call sizes, not
# ring stalls: 2.42ns/desc @1024, 3.31 @2048, 3.85 @4096.)
DSCRATCH = int(os.environ.get("GNN_DSCRATCH", "16384"))
OHG = int(os.environ.get("GNN_OHG", "2"))       # blocks per one-hot group
STREAM0 = os.environ.get("GNN_STREAM0", "1") == "1"  # layer-0 HWDGE stream
NPREP = int(os.environ.get("GNN_PREP", "0"))    # prepare-ahead calls per pass
# host fp8 one-hots via DMA: "1" = all layers, "l0" = layer 0 only, "0" = off.
# BOTH measured SLOWER than DVE is_eq ("1": 1192us vs 1094 baseline; "l0" on
# the Scalar engine: 819us vs 783 EXACT) - HWDGE desc-gen of the oh streams
# (~880ns per DIRECT2D) plus DMA-ordering stalls beat the is_eq cost even in
# the is_eq-heavy l0 phase.  Keep the path for experiments only.
OHMODE = os.environ.get("GNN_OHSTREAM", "0")
OHSTREAM = OHMODE in ("1", "l0")
# EXACT: one gather call per (block, part) chunk with pad indices = -1 and
# num_idxs_reg = this core's ACTUAL edge count (runtime register).  The Q7
# gather kernel trims trailing negatives, so each core only generates
# descriptors for its real edges (saves the cross-core padding, ~13%, on both
# desc-gen and DMA-engine time).  The register must match the trimmed count:
# the sequencer reserves ring space from num_idxs_reg while the Q7 writes the
# trimmed count - a static reg with -1 pads desyncs the ring (measured: device
# crash).
EXACT = os.environ.get("GNN_EXACT", "1") == "1"
FP8 = ml_dtypes.float8_e4m3fn


class Cfg:
    def __init__(self, n_nodes, n_edges, in_f, hid, out_f, n_cores, blocks_per_core):
        self.n_nodes = n_nodes
        self.n_edges = n_edges
        self.in_f = in_f
        self.hid = hid
        self.out_f = out_f
        self.out_p = 64
        self.n_cores = n_cores
        self.nb = blocks_per_core
        self.nb1 = (blocks_per_core + 1) // 2       # blocks in part1
        self.nb2 = blocks_per_core - self.nb1       # blocks in part2
        self.nblk = n_cores * blocks_per_core
        self.slots_core = blocks_per_core * P
        self.slots_tot = self.nblk * P
        self.p1_rows = n_cores * self.nb1 * P
        self.p2_rows = n_cores * self.nb2 * P
        assert self.p1_rows < 2**15 and self.p2_rows < 2**15


FULL_CFG = Cfg(n_nodes=50000, n_edges=800000, in_f=128, hid=128, out_f=47,
               n_cores=8, blocks_per_core=49)


def _ru128(x):
    return (int(x) + 127) // 128 * 128


class Call:
    def __init__(self, b0, nblocks, t0, T):
        self.b0 = b0          # first block in this call
        self.nblocks = nblocks
        self.t0 = t0          # subtile offset within the part strip
        self.T = T            # subtiles in this call


class EdgePlan:
    """Static per-(block, part) gather layout, identical across cores.

    Columns are laid out part-major: the part-0 strip (all blocks in order)
    followed by the part-1 strip, so merged gather calls / batched one-hot
    builds read contiguous column ranges.
    """

    def __init__(self, caps):
        nb = len(caps)
        self.nb = nb
        self.caps = caps
        self.tsub = [[caps[b][p] // P for p in range(2)] for b in range(nb)]
        self.tcol = [[0] * nb, [0] * nb]   # subtile offset of block within part strip
        self.Tpart = [0, 0]
        for p in range(2):
            t = 0
            for b in range(nb):
                self.tcol[p][b] = t
                t += self.tsub[b][p]
            self.Tpart[p] = t
        self.pbase = [0, self.Tpart[0]]
        self.dcols = self.Tpart[0] + self.Tpart[1]
        self.icols = self.dcols * 8        # int16 idx cols (16 rows per col)

        # gather calls.  EXACT: one call per (block, part) chunk, so each
        # chunk's (cross-core max) padding is TRAILING within its call and the
        # Q7's trailing-negative trim skips it.  Otherwise: uniform
        # MAXT-subtile ranges of the part strip (blocks may span calls).
        self.calls = [[], []]
        if EXACT:
            for p in range(2):
                for b in range(nb):
                    if self.tsub[b][p] > 0:
                        self.calls[p].append(
                            Call(b, 1, self.tcol[p][b], self.tsub[b][p]))
            self.maxt = max((c.T for cs in self.calls for c in cs), default=1)
        else:
            self.maxt = MAXN // P
            for p in range(2):
                t = 0
                while t < self.Tpart[p]:
                    T = min(self.maxt, self.Tpart[p] - t)
                    self.calls[p].append(Call(-1, 0, t, T))
                    t += T

        # subtile -> (call index, offset within call) per part
        self.call_of_t = [dict(), dict()]
        for p in range(2):
            for ci, c in enumerate(self.calls[p]):
                for off in range(c.T):
                    self.call_of_t[p][c.t0 + off] = (ci, off)

        # one-hot groups: OHG consecutive blocks
        self.groups = [[], []]
        for p in range(2):
            b = 0
            while b < nb:
                nbk = min(OHG, nb - b)
                t0 = self.tcol[p][b]
                T = sum(self.tsub[b + i][p] for i in range(nbk))
                self.groups[p].append(Call(b, nbk, t0, T))
                b += nbk
        self.oht = max(g.T for gs in self.groups for g in gs)


def prep_host(cfg: Cfg, x, src, dst):
    N, NBLK, C = cfg.n_nodes, cfg.nblk, cfg.n_cores
    F = cfg.in_f
    deg = np.bincount(dst, minlength=N).astype(np.int64)
    deg_inv = (1.0 / np.maximum(deg, 1)).astype(np.float32)

    # Degree-sorted snake deal -> balanced block loads, <=128 slots/block.
    order = np.argsort(-deg, kind="stable")
    k = np.arange(N)
    s, r = k // NBLK, k % NBLK
    blk_for_rank = np.where(s % 2 == 0, r, NBLK - 1 - r)
    blk_of_node = np.empty(N, np.int64)
    blk_of_node[order] = blk_for_rank
    slot_of_node = np.empty(N, np.int64)
    slot_of_node[order] = s
    assert slot_of_node.max() < P

    pos_of_node = blk_of_node * P + slot_of_node
    node_of_pos = np.full(cfg.slots_tot, -1, np.int64)
    node_of_pos[pos_of_node] = np.arange(N)

    # Source position in part-table coordinates
    n_core = blk_of_node // cfg.nb      # owning core
    n_bl = blk_of_node % cfg.nb         # block-local index
    n_part = (n_bl >= cfg.nb1).astype(np.int64)
    n_blp = np.where(n_part == 0, n_bl, n_bl - cfg.nb1)
    nbp = np.array([cfg.nb1, cfg.nb2])
    n_pos_p = (n_core * nbp[n_part] + n_blp) * P + slot_of_node

    # Edge keys: (dst block, src part)
    e_blk = blk_of_node[dst]
    e_part = n_part[src]
    key = e_blk * 2 + e_part
    e_order = np.argsort(key, kind="stable")
    cnt = np.bincount(key, minlength=NBLK * 2).reshape(NBLK, 2)
    cnt_c = cnt.reshape(C, cfg.nb, 2)
    caps = [[_ru128(cnt_c[:, b, part].max()) for part in range(2)]
            for b in range(cfg.nb)]
    plan = EdgePlan(caps)

    starts = np.zeros(NBLK * 2 + 1, np.int64)
    np.cumsum(cnt.reshape(-1), out=starts[1:])
    rank = np.arange(cfg.n_edges) - starts[key[e_order]]

    es, ed = src[e_order], dst[e_order]
    eb, ep = e_blk[e_order], e_part[e_order]
    e_core = eb // cfg.nb
    e_bl = eb % cfg.nb

    tcol_arr = np.array(plan.tcol)      # [2, nb]
    pbase = np.array(plan.pbase)        # [2]
    scol = pbase[ep] + tcol_arr[ep, e_bl]          # subtile base of (b, p)
    dcol = scol + rank // P
    jj = dcol * P + rank % P                       # flat strip position

    dstloc = np.full((C, P, plan.dcols), float(P), np.float32)
    dstloc[e_core, rank % P, dcol] = slot_of_node[ed].astype(np.float32)
    dstloc = dstloc.astype(BF16)

    # host-built fp8 one-hot strip: ohs[p, tg*P + s] = (edge at (p, tg) has
    # dst slot s); pad columns stay all-zero so pad gathers are killed.
    ohs = np.zeros((C, P, plan.dcols * P), FP8)
    ohs[e_core, rank % P, dcol * P + slot_of_node[ed]] = 1.0

    # int16 idx, wrapped in 16 partitions.  EXACT: pad = -1 so the Q7 gather
    # kernel's trailing-negative trim skips pad descriptors per-core (calls
    # are chunk-aligned so pads are trailing).  Otherwise pad = 0 (gathers
    # row 0, killed by the one-hot since pad dstloc stays == 128).
    pad_idx = -1 if EXACT else 0
    idxw = np.full((C, 16, plan.icols), pad_idx, np.int16)
    idxw[e_core, jj % 16, jj // 16] = n_pos_p[es].astype(np.int16)
    idx_arr = np.ascontiguousarray(np.tile(idxw, (1, 8, 1)))

    # per-core actual counts per (part, block), for EXACT num_idxs_reg
    percnt = np.zeros((C, 1, 2 * cfg.nb), np.int32)
    for p_ in range(2):
        percnt[:, 0, p_ * cfg.nb:(p_ + 1) * cfg.nb] = cnt_c[:, :, p_]

    # flat per-strip source index, for the host-side layer-0 pregather
    idxflat = np.zeros((C, plan.dcols * P), np.int64)
    idxflat[e_core, jj] = n_pos_p[es]

    # per-(slot, block) deg_inv and its block-diagonal form (for the fused
    # transpose+scale matmul)
    dinv_slot = np.ones((C, P, cfg.nb), np.float32)
    vpos = np.nonzero(node_of_pos >= 0)[0]
    vcore = vpos // cfg.slots_core
    vloc = vpos % cfg.slots_core
    dinv_slot[vcore, vloc % P, vloc // P] = deg_inv[node_of_pos[vpos]]
    ddiag = np.zeros((C, P, cfg.nb * P), np.float32)
    ss = np.arange(P)
    for b in range(cfg.nb):
        ddiag[:, ss, b * P + ss] = dinv_slot[:, ss, b]
    ddiag = ddiag.astype(BF16)

    # x in part layout + layer-0 msgs pregather (exact device tile layout)
    x_perm = np.zeros((cfg.slots_tot, F), np.float32)
    x_perm[pos_of_node] = x
    xp1 = np.zeros((cfg.p1_rows, F), np.float32)
    xp2 = np.zeros((cfg.p2_rows, F), np.float32)
    m1 = n_part[np.arange(N)] == 0
    xp1[n_pos_p[m1]] = x[m1]
    xp2[n_pos_p[~m1]] = x[~m1]

    T0, T1 = plan.Tpart
    flat0 = idxflat[:, :T0 * P].reshape(C, T0, P)
    flat1 = (idxflat[:, T0 * P:].reshape(C, T1, P))
    l0m1 = xp1[flat0].transpose(0, 2, 1, 3).reshape(C, P, T0 * F).astype(BF16)
    l0m2 = xp2[flat1].transpose(0, 2, 1, 3).reshape(C, P, T1 * F).astype(BF16)

    xT = np.ascontiguousarray(
        x_perm.reshape(C, cfg.slots_core, F).transpose(0, 2, 1)).astype(BF16)

    return dict(plan=plan, idx=idx_arr, dstloc=dstloc, ddiag=ddiag, ohs=ohs,
                percnt=percnt, l0m1=l0m1, l0m2=l0m2, xT=xT,
                node_of_pos=node_of_pos,
                xp1=xp1.astype(BF16), xp2=xp2.astype(BF16))


# --------------------------------------------------------------------------
# Device program
# --------------------------------------------------------------------------

def build_program(cfg: Cfg, plan: EdgePlan):
    import concourse.bacc as bacc
    import concourse.tile as tile
    from concourse import bass, mybir
    from concourse.library_config import mlp

    f32 = mybir.dt.float32
    bf16 = mybir.dt.bfloat16
    i16 = mybir.dt.int16
    AF = mybir.ActivationFunctionType
    OP = mybir.AluOpType

    NB, NB1, S = cfg.nb, cfg.nb1, cfg.slots_core
    HID, OUTP = cfg.hid, cfg.out_p
    NQ = 4
    MAXT = plan.maxt
    # single_packet by the ACTUAL max call size (EXACT calls can exceed MAXN)
    sp_eff = (_SP == "1") or (_SP == "auto" and MAXT * P <= 1024)

    nc = bacc.Bacc("TRN2", target_bir_lowering=False, debug=False,
                   enable_asserts=False, num_devices=cfg.n_cores,
                   num_swdge_queues=NQ,
                   dynamic_dma_scratch_size=DSCRATCH)

    if STREAM0:
        l0m1_d = nc.dram_tensor("l0m1", [P, plan.Tpart[0] * HID], bf16,
                                kind="ExternalInput").ap()
        l0m2_d = nc.dram_tensor("l0m2", [P, plan.Tpart[1] * HID], bf16,
                                kind="ExternalInput").ap()
    else:
        l0m1_d = nc.dram_tensor("xp1", [cfg.p1_rows, HID], bf16,
                                kind="ExternalInput").ap()
        l0m2_d = nc.dram_tensor("xp2", [cfg.p2_rows, HID], bf16,
                                kind="ExternalInput").ap()
    xtloc = nc.dram_tensor("xtloc", [HID, S], bf16, kind="ExternalInput").ap()
    idx_d = nc.dram_tensor("idx", [P, plan.icols], i16,
                           kind="ExternalInput").ap()
    dstloc_d = nc.dram_tensor("dstloc", [P, plan.dcols], bf16,
                              kind="ExternalInput").ap()
    ddiag_d = nc.dram_tensor("ddiag", [P, NB * P], bf16,
                             kind="ExternalInput").ap()
    if OHSTREAM:
        f8 = mybir.dt.float8e4
        ohs_d = nc.dram_tensor("ohs", [P, plan.dcols * P], f8,
                               kind="ExternalInput").ap()
    if EXACT:
        percnt_d = nc.dram_tensor("percnt", [1, 2 * NB], mybir.dt.int32,
                                  kind="ExternalInput").ap()
    w_d = {}
    for name, shp, dt_ in [("wl0", [cfg.in_f, HID], bf16),
                           ("wr0", [cfg.in_f, HID], bf16),
                           ("wl1", [HID, HID], bf16), ("wr1", [HID, HID], bf16),
                           ("wl2", [HID, OUTP], bf16), ("wr2", [HID, OUTP], bf16),
                           ("bl0t", [HID, 1], f32), ("bl1t", [HID, 1], f32),
                           ("bl2r", [1, OUTP], f32),
                           ("iota", [P, P], bf16), ("identb", [P, P], bf16),
                           ("ones", [1, P], f32)]:
        w_d[name] = nc.dram_tensor(name, shp, dt_, kind="ExternalInput").ap()
    out_d = nc.dram_tensor("out", [OUTP, S], f32, kind="ExternalOutput").ap()

    qctr = [0]

    with tile.TileContext(nc) as tc, nc.allow_low_precision("tol 2e-2"):
        with tc.tile_critical():
            nc.gpsimd.load_library(mlp)
        with (
            tc.tile_pool(name="const", bufs=1) as cp,
            tc.tile_pool(name="hT", bufs=2) as hTp,
            tc.tile_pool(name="msgs",
                         bufs=max(3, 24576 // (MAXT * P * 2))) as mp,
            tc.tile_pool(name="prep", bufs=max(NPREP, 1)) as ppool,
            tc.tile_pool(name="oh", bufs=5) as ohp,
            tc.tile_pool(name="sb", bufs=3) as sbp,
            tc.tile_pool(name="pagg", bufs=3, space="PSUM") as pagg,
            tc.tile_pool(name="ptr", bufs=2, space="PSUM") as ptr,
            tc.tile_pool(name="pd", bufs=2, space="PSUM") as pd,
            tc.tile_pool(name="dram", bufs=2, space="DRAM") as dp,
        ):
            def load_const(ap_, shp, dtype=f32, tag=None):
                t = cp.tile(shp, dtype, tag=tag or ap_.tensor.name)
                nc.sync.dma_start(out=t[:], in_=ap_)
                return t

            idx_sb = load_const(idx_d, [P, plan.icols], i16)
            dst_sb = load_const(dstloc_d, [P, plan.dcols], bf16)
            ddiag_sb = load_const(ddiag_d, [P, NB * P], bf16)
            iota_sb = load_const(w_d["iota"], [P, P], bf16)
            identb_sb = load_const(w_d["identb"], [P, P], bf16)
            ones_sb = load_const(w_d["ones"], [1, P])
            wsb = {}
            for k_ in ("wl0", "wr0", "wl1", "wr1", "wl2", "wr2"):
                wsb[k_] = load_const(w_d[k_], list(w_d[k_].shape), bf16)
            for k_ in ("bl0t", "bl1t", "bl2r"):
                wsb[k_] = load_const(w_d[k_], list(w_d[k_].shape), f32)

            hT_x = hTp.tile([P, S], bf16, tag="hT", name="hT_x")
            nc.sync.dma_start(out=hT_x[:], in_=xtloc)

            if EXACT:
                cnt_sb = load_const(percnt_d, [1, 2 * NB], mybir.dt.int32)
                cnt_reg = nc.alloc_register(mybir.EngineType.Pool, "gnncnt")

            PSEMS = ([nc.alloc_semaphore(f"gnnpq{q}") for q in range(NQ)]
                     if NPREP > 0 else None)
            PQC = [0] * NQ   # per-queue cumulative prep count

            iota_b = iota_sb[:].rearrange("p (a c) -> p a c", a=1)

            prep_state = {}

            def emit_preps(pass_id, part, source, nprep):
                """Generate descriptors for the first calls of a FUTURE pass
                (prepare_only): the Q7 does the gen work now, during idle
                time; the DMAs fire later via trigger_dma."""
                pb = plan.pbase[part]
                lst = []
                for k in range(min(nprep, len(plan.calls[part]))):
                    call = plan.calls[part][k]
                    t = ppool.tile([P, plan.maxt * HID], bf16, tag="prep")
                    q = qctr[0] % NQ
                    qctr[0] += 1
                    if EXACT:
                        # ring accounting must match the Q7's trailing-neg trim
                        nc.reg_load(cnt_reg,
                                    cnt_sb[0:1, part * NB + call.b0:
                                           part * NB + call.b0 + 1])
                        nreg = cnt_reg
                    else:
                        nreg = call.T * P
                    nc.gpsimd.dma_gather(
                        out_ap=t[:, :call.T * HID]
                            .rearrange("p (t c) -> p t c", c=HID),
                        in_ap=source,
                        idxs_ap=idx_sb[:, (pb + call.t0) * 8:
                                       (pb + call.t0 + call.T) * 8],
                        num_idxs=call.T * P,
                        num_idxs_reg=nreg,
                        elem_size=HID,
                        single_packet=sp_eff,
                        queue_num=q,
                        prepare_only=True,
                        sem=PSEMS[q],
                    )
                    PQC[q] += 1
                    lst.append((t, q))
                # consumers must wait for the queue's whole batch: per-engine
                # completion increments interleave across calls on a queue
                qtot = list(PQC)
                prep_state[pass_id] = [(t, q, qtot[q]) for (t, q) in lst]

            def emit_pass(part, source, is_stream, per_block, pass_id=None,
                          prefetch=None, blo=0, bhi=None):
                """Emit gathers/streams + one-hot builds + agg matmuls for
                blocks [blo, bhi) of `part` (group-aligned); call
                per_block(b, agg_ps) on each result."""
                pb = plan.pbase[part]
                calls = plan.calls[part]
                MAXT = plan.maxt
                mtiles = {}
                ci = [0]
                if bhi is None:
                    bhi = plan.nb
                if blo > 0:
                    # skip calls entirely before this block range (a call
                    # straddling the boundary is re-issued, which is harmless)
                    ts = plan.tcol[part][blo]
                    while (ci[0] < len(calls)
                           and calls[ci[0]].t0 + calls[ci[0]].T <= ts):
                        ci[0] += 1
                preps = prep_state.pop(pass_id, [])
                trig = set()

                def issue_thru(t_end):
                    while ci[0] < len(calls) and calls[ci[0]].t0 < t_end:
                        call = calls[ci[0]]
                        if ci[0] < len(preps):
                            t, q, qtot = preps[ci[0]]
                            if q not in trig:
                                nc.gpsimd.trigger_dma(count=None, queue_num=q)
                                trig.add(q)
                            # gate the consuming matmuls (PE) on the DMA
                            # completion sem — Tile only orders them after
                            # the prep's dispatch, not the data landing
                            nc.tensor.wait_ge(PSEMS[q], 16 * qtot)
                            mtiles[ci[0]] = t
                            ci[0] += 1
                            continue
                        msgs = mp.tile([P, MAXT * HID], bf16, tag="msgs")
                        mv = msgs[:, :call.T * HID]
                        if is_stream:
                            # alternate HWDGE engines: each DIRECT2D dispatch
                            # costs ~640ns of sequencer time (104us on Sync in
                            # l0 alone) - spread across Sync and Scalar
                            eng = nc.sync if (ci[0] & 1) else nc.scalar
                            eng.dma_start(
                                out=mv,
                                in_=source[:, call.t0 * HID:
                                           (call.t0 + call.T) * HID])
                        else:
                            if EXACT:
                                cid = part * NB + call.b0
                                nc.reg_load(
                                    cnt_reg, cnt_sb[0:1, cid:cid + 1])
                                nreg = cnt_reg
                            else:
                                nreg = call.T * P
                            nc.gpsimd.dma_gather(
                                out_ap=mv.rearrange("p (t c) -> p t c", c=HID),
                                in_ap=source,
                                idxs_ap=idx_sb[:, (pb + call.t0) * 8:
                                               (pb + call.t0 + call.T) * 8],
                                num_idxs=call.T * P,
                                num_idxs_reg=nreg,
                                elem_size=HID,
                                single_packet=sp_eff,
                                queue_num=qctr[0] % NQ,
                            )
                            qctr[0] += 1
                        mtiles[ci[0]] = msgs
                        ci[0] += 1

                oh_dma = OHMODE == "1" or (OHMODE == "l0" and is_stream)
                for g in [g for g in plan.groups[part]
                          if blo <= g.b0 < bhi]:
                    if oh_dma:
                        O = ohp.tile([P, plan.oht * P], mybir.dt.float8e4,
                                     tag="oh")
                        nc.scalar.dma_start(
                            out=O[:, :g.T * P],
                            in_=ohs_d[:, (pb + g.t0) * P:
                                      (pb + g.t0 + g.T) * P])
                    else:
                        # fp8 output halves the DVE write bytes; iota as in0
                        # gives the contiguous inner stride.  (0/1 are exact
                        # in fp8e4; the PE takes mixed fp8xbf16 operands.)
                        O = ohp.tile([P, plan.oht * P], mybir.dt.float8e4,
                                     tag="oh")
                        nc.vector.tensor_tensor(
                            out=O[:, :g.T * P]
                                .rearrange("p (t c) -> p t c", t=g.T),
                            in0=iota_b.to_broadcast([P, g.T, P]),
                            in1=dst_sb[:, pb + g.t0:pb + g.t0 + g.T]
                                .to_broadcast([P, g.T, P]),
                            op=OP.is_equal,
                        )
                    for b in range(g.b0, g.b0 + g.nblocks):
                        tn = plan.tsub[b][part]
                        if tn == 0:
                            per_block(b, None)
                            continue
                        t0b = plan.tcol[part][b]
                        issue_thru(t0b + tn)
                        go = t0b - g.t0            # subtile offset in O
                        agg_ps = pagg.tile([P, HID], f32, tag="agg")
                        for t in range(tn):
                            tg = t0b + t
                            k, off = plan.call_of_t[part][tg]
                            nc.tensor.matmul(
                                out=agg_ps[:],
                                lhsT=O[:, (go + t) * P:(go + t + 1) * P],
                                rhs=mtiles[k][:, off * HID:(off + 1) * HID],
                                start=(t == 0), stop=(t == tn - 1),
                            )
                        per_block(b, agg_ps)
                if prefetch is not None:
                    emit_preps(*prefetch)

            def pass_a(part, source, is_stream, aggbuf, pass_id=None,
                       prefetch=None, blo=0, bhi=None):
                def pb_(b, agg_ps):
                    if agg_ps is None:
                        nc.vector.memset(aggbuf[:, b * P:(b + 1) * P], 0)
                    else:
                        nc.scalar.activation(aggbuf[:, b * P:(b + 1) * P],
                                             agg_ps[:], AF.Copy)
                emit_pass(part, source, is_stream, pb_, pass_id, prefetch,
                          blo, bhi)

            def pass_b(li, part, source, is_stream, aggbuf, wl, wr, bias,
                       hT_prev, hT_cur, bounce1, bounce2, pass_id=None,
                       prefetch=None, blo=0, bhi=None):
                K = OUTP if li == 2 else HID

                def pb_(b, agg_ps):
                    bs = slice(b * P, (b + 1) * P)
                    mean_pre = sbp.tile([P, HID], bf16, tag="mean_pre")
                    if agg_ps is not None:
                        nc.vector.tensor_tensor(
                            out=mean_pre[:], in0=aggbuf[:, bs],
                            in1=agg_ps[:], op=OP.add)
                    else:
                        nc.vector.tensor_copy(out=mean_pre[:], in_=aggbuf[:, bs])
                    # fused transpose+scale: meanT = mean_pre^T @ diag(dinv_b)
                    mt_ps = ptr.tile([P, P], f32, tag="mt")
                    nc.tensor.matmul(out=mt_ps[:], lhsT=mean_pre[:],
                                     rhs=ddiag_sb[:, bs], start=True, stop=True)
                    meanT = sbp.tile([P, P], bf16, tag="meanT")
                    nc.scalar.activation(meanT[:], mt_ps[:], AF.Copy)

                    dens = pd.tile([K, P], f32, tag="dens")
                    nc.tensor.matmul(out=dens[:], lhsT=wl[:, :K], rhs=meanT[:],
                                     start=True, stop=False)
                    if li == 2:
                        nc.tensor.matmul(out=dens[:], lhsT=wr[:, :K],
                                         rhs=hT_prev[:, bs],
                                         start=False, stop=False)
                        nc.tensor.matmul(out=dens[:], lhsT=bias[:],
                                         rhs=ones_sb[:], start=False, stop=True)
                        out_sb = sbp.tile([OUTP, P], f32, tag="out_sb")
                        nc.scalar.activation(out_sb[:], dens[:], AF.Copy)
                        nc.sync.dma_start(out=out_d[:, bs], in_=out_sb[:])
                    else:
                        nc.tensor.matmul(out=dens[:], lhsT=wr[:, :K],
                                         rhs=hT_prev[:, bs],
                                         start=False, stop=True)
                        nc.scalar.activation(hT_cur[:, bs], dens[:], AF.Relu,
                                             bias=bias[:])
                        h_ps = ptr.tile([P, P], bf16, tag="hps", bufs=1)
                        nc.tensor.transpose(h_ps[:], hT_cur[:, bs], identb_sb[:])
                        h_b = sbp.tile([P, HID], bf16, tag="h_b")
                        nc.scalar.activation(h_b[:], h_ps[:], AF.Copy)
                        beng = nc.sync if (b & 1) else nc.scalar
                        if b < NB1:
                            beng.dma_start(
                                out=bounce1[b * P:(b + 1) * P, :], in_=h_b[:])
                        else:
                            bb = b - NB1
                            beng.dma_start(
                                out=bounce2[bb * P:(bb + 1) * P, :], in_=h_b[:])
                emit_pass(part, source, is_stream, pb_, pass_id, prefetch,
                          blo, bhi)

            def allgather(bounce, full):
                nc.gpsimd.collective_compute(
                    "AllGather", OP.bypass,
                    replica_groups=[list(range(cfg.n_cores))],
                    ins=[bounce.opt()], outs=[full.opt()],
                )

            # DRAM staging (double buffered across layers)
            tb1 = [dp.tile([cfg.p1_rows, HID], bf16, tag="t1",
                           addr_space="Shared", name=f"t1_{i}") for i in range(2)]
            tb2 = [dp.tile([cfg.p2_rows, HID], bf16, tag="t2",
                           addr_space="Shared", name=f"t2_{i}") for i in range(2)]
            bn1 = [dp.tile([cfg.nb1 * P, HID], bf16, tag="b1", name=f"b1_{i}")
                   for i in range(2)]
            bn2 = [dp.tile([cfg.nb2 * P, HID], bf16, tag="b2", name=f"b2_{i}")
                   for i in range(2)]

            hT = [hTp.tile([P, S], bf16, tag="hT", name=f"hT{i}")
                  for i in range(2)]
            ab = [hTp.tile([P, S], bf16, tag="aggbuf", name=f"ab{i}")
                  for i in range(2)]

            # Prefetch chain: each pass generates (prepare_only) the gather
            # descriptors for the FIRST NPREP calls of the NEXT gather pass
            # at the end of its own emission, so the Q7 gen work overlaps
            # the AllGather wait / compute tail. Fired via trigger_dma.
            def pf(pid, part, src):
                return (pid, part, src) + (NPREP,) if NPREP > 0 else None

            def prep_barrier(tbl):
                # The preps are emitted before the table's AllGather exists
                # in the IR, so no RAW dep reaches them or their triggers.
                # This gpsimd read of the table blocks the engine (and hence
                # the triggers behind it in FIFO order) until the data lands.
                if NPREP > 0:
                    bsc = sbp.tile([1, HID], bf16, tag="bar")
                    nc.sync.dma_start(out=bsc[:], in_=tbl[0:1, 0:HID])
                    bdst = sbp.tile([16, HID], bf16, tag="bar2")
                    nc.gpsimd.partition_broadcast(bdst[:], bsc[:], channels=16)

            # layer 0 (host-pregathered msgs, streamed via HWDGE)
            # (a two-half split to launch bounce1's AllGather earlier was
            # measured SLOWER — 1083 vs 1048 us — the collective dispatch
            # is dep-driven and already early; the split only disrupted
            # cross-pass pipelining)
            pass_a(0, l0m1_d, STREAM0, ab[0])
            pass_b(0, 1, l0m2_d, STREAM0, ab[0], wsb["wl0"], wsb["wr0"],
                   wsb["bl0t"], hT_x, hT[0], bn1[0][:], bn2[0][:],
                   prefetch=pf("l1p1", 0, tb1[0][:]))
            allgather(bn1[0], tb1[0])
            allgather(bn2[0], tb2[0])

            # layer 1
            prep_barrier(tb1[0])
            pass_a(0, tb1[0][:], False, ab[1], pass_id="l1p1",
                   prefetch=pf("l1p2", 1, tb2[0][:]))
            prep_barrier(tb2[0])
            pass_b(1, 1, tb2[0][:], False, ab[1], wsb["wl1"], wsb["wr1"],
                   wsb["bl1t"], hT[0], hT[1], bn1[1][:], bn2[1][:],
                   pass_id="l1p2", prefetch=pf("l2p1", 0, tb1[1][:]))
            allgather(bn1[1], tb1[1])
            allgather(bn2[1], tb2[1])

            # layer 2
            prep_barrier(tb1[1])
            pass_a(0, tb1[1][:], False, ab[0], pass_id="l2p1",
                   prefetch=pf("l2p2", 1, tb2[1][:]))
            prep_barrier(tb2[1])
            pass_b(2, 1, tb2[1][:], False, ab[0], wsb["wl2"], wsb["wr2"],
                   wsb["bl2r"], hT[1], None, None, None, pass_id="l2p2")

    nc.compile()
    return nc


# --------------------------------------------------------------------------
# Entry point
# --------------------------------------------------------------------------

def _make_in_maps(cfg: Cfg, host, weights):
    iota = np.broadcast_to(np.arange(P, dtype=np.float32), (P, P)).astype(BF16)
    ident = np.eye(P, dtype=np.float32)
    ones = np.ones((1, P), np.float32)
    maps = []
    for i in range(cfg.n_cores):
        m = dict(
            xtloc=host["xT"][i],
            idx=host["idx"][i],
            dstloc=host["dstloc"][i],
            ddiag=host["ddiag"][i],
            iota=iota, identb=ident.astype(BF16), ones=ones,
        )
        if OHSTREAM:
            m["ohs"] = host["ohs"][i]
        if EXACT:
            m["percnt"] = host["percnt"][i]
        if STREAM0:
            m["l0m1"] = host["l0m1"][i]
            m["l0m2"] = host["l0m2"][i]
        else:
            m["xp1"] = host["xp1"]
            m["xp2"] = host["xp2"]
        m.update(weights)
        maps.append(m)
    return maps


def _pad_w(w, outp):
    w = np.asarray(w, np.float32)
    if w.shape[-1] < outp:
        pad = np.zeros(w.shape[:-1] + (outp - w.shape[-1],), np.float32)
        w = np.concatenate([w, pad], axis=-1)
    return w


def _ensure_ntff_hook():
    import types

    try:
        from antenv.axon_hooks import get_axon_ntff_profile_hook  # noqa: F401
        return
    except ImportError:
        pass
    try:
        import antenv
        from trn_agent_boot.trn_boot import _ntff_profile_via_ctypes
    except ImportError:
        return
    hook = _ntff_profile_via_ctypes("/opt/axon/libaxon_pjrt.so")
    mod = types.ModuleType("antenv.axon_hooks")
    mod.get_axon_ntff_profile_hook = lambda: hook
    mod.set_axon_ntff_profile_hook = lambda h: None
    sys.modules["antenv.axon_hooks"] = mod
    antenv.axon_hooks = mod


def run(cfg: Cfg, inputs, trace=False):
    import concourse.bass_utils as bu
    from concourse.bass_utils import run_bass_kernel_spmd

    if trace:
        _ensure_ntff_hook()
        bu.upload_artifacts = lambda d: str(d)

    x = np.asarray(inputs["x"], np.float32)
    ei = np.asarray(inputs["edge_index"])
    src = ei[0].astype(np.int64)
    dst = ei[1].astype(np.int64)

    host = prep_host(cfg, x, src, dst)
    weights = dict(
        wl0=np.asarray(inputs["Wl0"], np.float32).astype(BF16),
        wr0=np.asarray(inputs["Wr0"], np.float32).astype(BF16),
        wl1=np.asarray(inputs["Wl1"], np.float32).astype(BF16),
        wr1=np.asarray(inputs["Wr1"], np.float32).astype(BF16),
        wl2=_pad_w(inputs["Wl2"], cfg.out_p).astype(BF16),
        wr2=_pad_w(inputs["Wr2"], cfg.out_p).astype(BF16),
        bl0t=np.asarray(inputs["bl0"], np.float32).reshape(-1, 1),
        bl1t=np.asarray(inputs["bl1"], np.float32).reshape(-1, 1),
        bl2r=_pad_w(np.asarray(inputs["bl2"], np.float32).reshape(1, -1),
                    cfg.out_p),
    )

    nc = build_program(cfg, host["plan"])
    in_maps = _make_in_maps(cfg, host, weights)
    res = run_bass_kernel_spmd(nc, in_maps, core_ids=list(range(cfg.n_cores)),
                               trace=trace)

    out_full = np.empty((cfg.n_nodes, cfg.out_f), np.float32)
    node_of_pos = host["node_of_pos"]
    for i in range(cfg.n_cores):
        o = np.asarray(res.results[i]["out"]).T    # [S, OUTP]
        pos = np.arange(i * cfg.slots_core, (i + 1) * cfg.slots_core)
        nodes = node_of_pos[pos]
        valid = nodes >= 0
        out_full[nodes[valid]] = o[valid][:, :cfg.out_f]
    return out_full, res


def kernel(**inputs) -> np.ndarray:
    trace = os.environ.get("GNN_TRACE", "0") == "1"
    out, _ = run(FULL_CFG, inputs, trace=trace)
    return out



# revision 40
# speedup vs baseline: 1.0648x; 1.0648x over previous
"""Distributed GraphSAGE (3-layer, mean aggregation) on 8 Trainium2 NeuronCores.

v3 (767us, from v2's 1087us): the measured wall is Q7 SWDGE descriptor
generation (~994ns/call + ~2.4ns/index, ~70% engine busy); DMA engines run at
~14ns/desc and are never the constraint.  v3 therefore attacks generated-
descriptor COUNT via EXACT mode: one gather call per (block, part) chunk, pad
indices are -1, and num_idxs_reg is a Pool-engine register loaded with this
core's actual edge count, so the Q7's trailing-negative trim skips all
cross-core padding (~13% of descriptors) at both desc-gen and DMA time.  The
register must equal the trimmed count: sequencer-side ring accounting follows
num_idxs_reg while the Q7 writes the trimmed stream - any mismatch desyncs
the descriptor ring (device crash).  One-hot tiles are fp8 (exact 0/1, mixed
fp8 x bf16 matmul is supported).  Measured dead ends kept behind env flags:
MAXN>1024 (per-index gen cost rises with call size), prepare_only/trigger
pipelining (data races -> NaN), DRAM-streamed host one-hots (HWDGE DIRECT2D
dispatch ~880ns each beats the DVE is_eq saving), spreading dma_starts onto
the Scalar sequencer (interferes with its copy/relu work).

Strategy (ClusterGCN-style node partitioning), v2:
  - Nodes are load-balance-packed into 128-slot "blocks" (degree-aware snake
    packing), 49 blocks per core -> each core owns 6272 node slots.
  - The gather table (h, bf16) is split into two parts: part1 = every core's
    blocks 0..24, part2 = blocks 25..48.  Each part is AllGathered as soon as
    its blocks finish, so collectives overlap compute, and each part stays
    below the int16 index range of dma_gather.
  - Per layer, two passes over the 49 destination blocks: pass A aggregates
    part1-sourced edges (partials parked in SBUF via the Scalar engine),
    pass B aggregates part2-sourced edges, combines, and runs the epilogue.
  - v2 changes vs v1 (which was SWDGE descriptor-generation bound):
    * Gather calls are MERGED across destination blocks (up to 8192 indices
      per dma_gather) to amortize the ~1-2us fixed SWDGE cost. Per-block
      edge caps are padded to %128 so block boundaries align with matmul
      subtiles; pad indices gather row 0 and are killed by the one-hot.
    * Layer-0 messages (x[src]) are pre-gathered on the host into the exact
      SBUF tile layout and streamed with HWDGE dma_start - no descriptor
      generation at all for layer 0.
    * One-hot builds (Vector engine is_equal) are batched over pairs of
      blocks to amortize DVE dispatch overhead.
    * The mean scaling is folded into the transpose matmul: meanT =
      mean_pre^T @ diag(deg_inv_block), replacing the Scalar-engine scale.
    * The dense epilogue is flipped: densT[K, slot] = Wl^T meanT + Wr^T hT,
      so bias+ReLU fuse into one Scalar activation (bias is per-partition)
      and h^T lands directly in the next layer's hT strip. This removes all
      bias matmuls and one transpose per block.
    * PSUM->SBUF copies moved from Vector to Scalar engine; weights / hT
      strips in bf16.
  - Output is produced transposed ([out_f, slots]); the host unshards.
"""

import sys

if "/opt/trn_rl_repo" not in sys.path:
    sys.path.insert(0, "/opt/trn_rl_repo")

import os
import numpy as np
import ml_dtypes

BF16 = ml_dtypes.bfloat16
P = 128
MAXN = int(os.environ.get("GNN_MAXN", "1024"))  # max indices per dma_gather
# single_packet mode for dma_gather: "auto" = True iff MAXN <= 1024
_SP = os.environ.get("GNN_SP", "auto")
SP = (MAXN <= 1024) if _SP == "auto" else (_SP == "1")
# SWDGE descriptor-ring carveout (bytes per partition).  Rings are per
# (engine, queue) partition rows: 16KiB/16B = 1024 descs per engine per queue
# (~15 calls deep at MAXN=1024) - the default is plenty.  (The MAXN=2048/4096
# regressions were intrinsic per-desc gen slowdown at larger call sizes, not
# ring stalls: 2.42ns/desc @1024, 3.31 @2048, 3.85 @4096.)
DSCRATCH = int(os.environ.get("GNN_DSCRATCH", "16384"))
OHG = int(os.environ.get("GNN_OHG", "2"))       # blocks per one-hot group
STREAM0 = os.environ.get("GNN_STREAM0", "1") == "1"  # layer-0 HWDGE stream
NPREP = int(os.environ.get("GNN_PREP", "0"))    # prepare-ahead calls per pass
# host fp8 one-hots via DMA: "1" = all layers, "l0" = layer 0 only, "0" = off.
# BOTH measured SLOWER than DVE is_eq ("1": 1192us vs 1094 baseline; "l0" on
# the Scalar engine: 819us vs 783 EXACT) - HWDGE desc-gen of the oh streams
# (~880ns per DIRECT2D) plus DMA-ordering stalls beat the is_eq cost even in
# the is_eq-heavy l0 phase.  Keep the path for experiments only.
OHMODE = os.environ.get("GNN_OHSTREAM", "0")
OHSTREAM = OHMODE in ("1", "l0")
# EXACT: one gather call per (block, part) chunk with pad indices = -1 and
# num_idxs_reg = this core's ACTUAL edge count (runtime register).  The Q7
# gather kernel trims trailing negatives, so each core only generates
# descriptors for its real edges (saves the cross-core padding, ~13%, on both
# desc-gen and DMA-engine time).  The register must match the trimmed count:
# the sequencer reserves ring space from num_idxs_reg while the Q7 writes the
# trimmed count - a static reg with -1 pads desyncs the ring (measured: device
# crash).
EXACT = os.environ.get("GNN_EXACT", "1") == "1"
FP8 = ml_dtypes.float8_e4m3fn


class Cfg:
    def __init__(self, n_nodes, n_edges, in_f, hid, out_f, n_cores, blocks_per_core):
        self.n_nodes = n_nodes
        self.n_edges = n_edges
        self.in_f = in_f
        self.hid = hid
        self.out_f = out_f
        self.out_p = 64
        self.n_cores = n_cores
        self.nb = blocks_per_core
        self.nb1 = (blocks_per_core + 1) // 2       # blocks in part1
        self.nb2 = blocks_per_core - self.nb1       # blocks in part2
        self.nblk = n_cores * blocks_per_core
        self.slots_core = blocks_per_core * P
        self.slots_tot = self.nblk * P
        self.p1_rows = n_cores * self.nb1 * P
        self.p2_rows = n_cores * self.nb2 * P
        assert self.p1_rows < 2**15 and self.p2_rows < 2**15


FULL_CFG = Cfg(n_nodes=50000, n_edges=800000, in_f=128, hid=128, out_f=47,
               n_cores=8, blocks_per_core=49)


def _ru128(x):
    return (int(x) + 127) // 128 * 128


class Call:
    def __init__(self, b0, nblocks, t0, T):
        self.b0 = b0          # first block in this call
        self.nblocks = nblocks
        self.t0 = t0          # subtile offset within the part strip
        self.T = T            # subtiles in this call


class EdgePlan:
    """Static per-(block, part) gather layout, identical across cores.

    Columns are laid out part-major: the part-0 strip (all blocks in order)
    followed by the part-1 strip, so merged gather calls / batched one-hot
    builds read contiguous column ranges.
    """

    def __init__(self, caps):
        nb = len(caps)
        self.nb = nb
        self.caps = caps
        self.tsub = [[caps[b][p] // P for p in range(2)] for b in range(nb)]
        self.tcol = [[0] * nb, [0] * nb]   # subtile offset of block within part strip
        self.Tpart = [0, 0]
        for p in range(2):
            t = 0
            for b in range(nb):
                self.tcol[p][b] = t
                t += self.tsub[b][p]
            self.Tpart[p] = t
        self.pbase = [0, self.Tpart[0]]
        self.dcols = self.Tpart[0] + self.Tpart[1]
        self.icols = self.dcols * 8        # int16 idx cols (16 rows per col)

        # gather calls.  EXACT: one call per (block, part) chunk, so each
        # chunk's (cross-core max) padding is TRAILING within its call and the
        # Q7's trailing-negative trim skips it.  Otherwise: uniform
        # MAXT-subtile ranges of the part strip (blocks may span calls).
        self.calls = [[], []]
        if EXACT:
            for p in range(2):
                for b in range(nb):
                    if self.tsub[b][p] > 0:
                        self.calls[p].append(
                            Call(b, 1, self.tcol[p][b], self.tsub[b][p]))
            self.maxt = max((c.T for cs in self.calls for c in cs), default=1)
        else:
            self.maxt = MAXN // P
            for p in range(2):
                t = 0
                while t < self.Tpart[p]:
                    T = min(self.maxt, self.Tpart[p] - t)
                    self.calls[p].append(Call(-1, 0, t, T))
                    t += T

        # subtile -> (call index, offset within call) per part
        self.call_of_t = [dict(), dict()]
        for p in range(2):
            for ci, c in enumerate(self.calls[p]):
                for off in range(c.T):
                    self.call_of_t[p][c.t0 + off] = (ci, off)

        # one-hot groups: OHG consecutive blocks
        self.groups = [[], []]
        for p in range(2):
            b = 0
            while b < nb:
                nbk = min(OHG, nb - b)
                t0 = self.tcol[p][b]
                T = sum(self.tsub[b + i][p] for i in range(nbk))
                self.groups[p].append(Call(b, nbk, t0, T))
                b += nbk
        self.oht = max(g.T for gs in self.groups for g in gs)


def prep_host(cfg: Cfg, x, src, dst):
    N, NBLK, C = cfg.n_nodes, cfg.nblk, cfg.n_cores
    F = cfg.in_f
    deg = np.bincount(dst, minlength=N).astype(np.int64)
    deg_inv = (1.0 / np.maximum(deg, 1)).astype(np.float32)

    # Degree-sorted snake deal -> balanced block loads, <=128 slots/block.
    order = np.argsort(-deg, kind="stable")
    k = np.arange(N)
    s, r = k // NBLK, k % NBLK
    blk_for_rank = np.where(s % 2 == 0, r, NBLK - 1 - r)
    blk_of_node = np.empty(N, np.int64)
    blk_of_node[order] = blk_for_rank
    slot_of_node = np.empty(N, np.int64)
    slot_of_node[order] = s
    assert slot_of_node.max() < P

    pos_of_node = blk_of_node * P + slot_of_node
    node_of_pos = np.full(cfg.slots_tot, -1, np.int64)
    node_of_pos[pos_of_node] = np.arange(N)

    # Source position in part-table coordinates
    n_core = blk_of_node // cfg.nb      # owning core
    n_bl = blk_of_node % cfg.nb         # block-local index
    n_part = (n_bl >= cfg.nb1).astype(np.int64)
    n_blp = np.where(n_part == 0, n_bl, n_bl - cfg.nb1)
    nbp = np.array([cfg.nb1, cfg.nb2])
    n_pos_p = (n_core * nbp[n_part] + n_blp) * P + slot_of_node

    # Edge keys: (dst block, src part)
    e_blk = blk_of_node[dst]
    e_part = n_part[src]
    key = e_blk * 2 + e_part
    e_order = np.argsort(key, kind="stable")
    cnt = np.bincount(key, minlength=NBLK * 2).reshape(NBLK, 2)
    cnt_c = cnt.reshape(C, cfg.nb, 2)
    caps = [[_ru128(cnt_c[:, b, part].max()) for part in range(2)]
            for b in range(cfg.nb)]
    plan = EdgePlan(caps)

    starts = np.zeros(NBLK * 2 + 1, np.int64)
    np.cumsum(cnt.reshape(-1), out=starts[1:])
    rank = np.arange(cfg.n_edges) - starts[key[e_order]]

    es, ed = src[e_order], dst[e_order]
    eb, ep = e_blk[e_order], e_part[e_order]
    e_core = eb // cfg.nb
    e_bl = eb % cfg.nb

    tcol_arr = np.array(plan.tcol)      # [2, nb]
    pbase = np.array(plan.pbase)        # [2]
    scol = pbase[ep] + tcol_arr[ep, e_bl]          # subtile base of (b, p)
    dcol = scol + rank // P
    jj = dcol * P + rank % P                       # flat strip position

    dstloc = np.full((C, P, plan.dcols), float(P), np.float32)
    dstloc[e_core, rank % P, dcol] = slot_of_node[ed].astype(np.float32)
    dstloc = dstloc.astype(BF16)

    # host-built fp8 one-hot strip: ohs[p, tg*P + s] = (edge at (p, tg) has
    # dst slot s); pad columns stay all-zero so pad gathers are killed.
    ohs = np.zeros((C, P, plan.dcols * P), FP8)
    ohs[e_core, rank % P, dcol * P + slot_of_node[ed]] = 1.0

    # int16 idx, wrapped in 16 partitions.  EXACT: pad = -1 so the Q7 gather
    # kernel's trailing-negative trim skips pad descriptors per-core (calls
    # are chunk-aligned so pads are trailing).  Otherwise pad = 0 (gathers
    # row 0, killed by the one-hot since pad dstloc stays == 128).
    pad_idx = -1 if EXACT else 0
    idxw = np.full((C, 16, plan.icols), pad_idx, np.int16)
    idxw[e_core, jj % 16, jj // 16] = n_pos_p[es].astype(np.int16)
    idx_arr = np.ascontiguousarray(np.tile(idxw, (1, 8, 1)))

    # per-core actual counts per (part, block), for EXACT num_idxs_reg
    percnt = np.zeros((C, 1, 2 * cfg.nb), np.int32)
    for p_ in range(2):
        percnt[:, 0, p_ * cfg.nb:(p_ + 1) * cfg.nb] = cnt_c[:, :, p_]

    # flat per-strip source index, for the host-side layer-0 pregather
    idxflat = np.zeros((C, plan.dcols * P), np.int64)
    idxflat[e_core, jj] = n_pos_p[es]

    # per-(slot, block) deg_inv and its block-diagonal form (for the fused
    # transpose+scale matmul)
    dinv_slot = np.ones((C, P, cfg.nb), np.float32)
    vpos = np.nonzero(node_of_pos >= 0)[0]
    vcore = vpos // cfg.slots_core
    vloc = vpos % cfg.slots_core
    dinv_slot[vcore, vloc % P, vloc // P] = deg_inv[node_of_pos[vpos]]
    ddiag = np.zeros((C, P, cfg.nb * P), np.float32)
    ss = np.arange(P)
    for b in range(cfg.nb):
        ddiag[:, ss, b * P + ss] = dinv_slot[:, ss, b]
    ddiag = ddiag.astype(BF16)

    # x in part layout + layer-0 msgs pregather (exact device tile layout)
    x_perm = np.zeros((cfg.slots_tot, F), np.float32)
    x_perm[pos_of_node] = x
    xp1 = np.zeros((cfg.p1_rows, F), np.float32)
    xp2 = np.zeros((cfg.p2_rows, F), np.float32)
    m1 = n_part[np.arange(N)] == 0
    xp1[n_pos_p[m1]] = x[m1]
    xp2[n_pos_p[~m1]] = x[~m1]

    T0, T1 = plan.Tpart
    flat0 = idxflat[:, :T0 * P].reshape(C, T0, P)
    flat1 = (idxflat[:, T0 * P:].reshape(C, T1, P))
    l0m1 = xp1[flat0].transpose(0, 2, 1, 3).reshape(C, P, T0 * F).astype(BF16)
    l0m2 = xp2[flat1].transpose(0, 2, 1, 3).reshape(C, P, T1 * F).astype(BF16)

    xT = np.ascontiguousarray(
        x_perm.reshape(C, cfg.slots_core, F).transpose(0, 2, 1)).astype(BF16)

    return dict(plan=plan, idx=idx_arr, dstloc=dstloc, ddiag=ddiag, ohs=ohs,
                percnt=percnt, l0m1=l0m1, l0m2=l0m2, xT=xT,
                node_of_pos=node_of_pos,
                xp1=xp1.astype(BF16), xp2=xp2.astype(BF16))


# --------------------------------------------------------------------------
# Device program
# --------------------------------------------------------------------------

def build_program(cfg: Cfg, plan: EdgePlan):
    import concourse.bacc as bacc
    import concourse.tile as tile
    from concourse import bass, mybir
    from concourse.library_config import mlp

    f32 = mybir.dt.float32
    bf16 = mybir.dt.bfloat16
    i16 = mybir.dt.int16
    AF = mybir.ActivationFunctionType
    OP = mybir.AluOpType

    NB, NB1, S = cfg.nb, cfg.nb1, cfg.slots_core
    HID, OUTP = cfg.hid, cfg.out_p
    NQ = 4
    MAXT = plan.maxt
    # single_packet by the ACTUAL max call size (EXACT calls can exceed MAXN)
    sp_eff = (_SP == "1") or (_SP == "auto" and MAXT * P <= 1024)

    nc = bacc.Bacc("TRN2", target_bir_lowering=False, debug=False,
                   enable_asserts=False, num_devices=cfg.n_cores,
                   num_swdge_queues=NQ,
                   dynamic_dma_scratch_size=DSCRATCH)

    if STREAM0:
        l0m1_d = nc.dram_tensor("l0m1", [P, plan.Tpart[0] * HID], bf16,
                                kind="ExternalInput").ap()
        l0m2_d = nc.dram_tensor("l0m2", [P, plan.Tpart[1] * HID], bf16,
                                kind="ExternalInput").ap()
    else:
        l0m1_d = nc.dram_tensor("xp1", [cfg.p1_rows, HID], bf16,
                                kind="ExternalInput").ap()
        l0m2_d = nc.dram_tensor("xp2", [cfg.p2_rows, HID], bf16,
                                kind="ExternalInput").ap()
    xtloc = nc.dram_tensor("xtloc", [HID, S], bf16, kind="ExternalInput").ap()
    idx_d = nc.dram_tensor("idx", [P, plan.icols], i16,
                           kind="ExternalInput").ap()
    dstloc_d = nc.dram_tensor("dstloc", [P, plan.dcols], bf16,
                              kind="ExternalInput").ap()
    ddiag_d = nc.dram_tensor("ddiag", [P, NB * P], bf16,
                             kind="ExternalInput").ap()
    if OHSTREAM:
        f8 = mybir.dt.float8e4
        ohs_d = nc.dram_tensor("ohs", [P, plan.dcols * P], f8,
                               kind="ExternalInput").ap()
    if EXACT:
        percnt_d = nc.dram_tensor("percnt", [1, 2 * NB], mybir.dt.int32,
                                  kind="ExternalInput").ap()
    w_d = {}
    for name, shp, dt_ in [("wl0", [cfg.in_f, HID], bf16),
                           ("wr0", [cfg.in_f, HID], bf16),
                           ("wl1", [HID, HID], bf16), ("wr1", [HID, HID], bf16),
                           ("wl2", [HID, OUTP], bf16), ("wr2", [HID, OUTP], bf16),
                           ("bl0t", [HID, 1], f32), ("bl1t", [HID, 1], f32),
                           ("bl2r", [1, OUTP], f32),
                           ("iota", [P, P], bf16), ("identb", [P, P], bf16),
                           ("ones", [1, P], f32)]:
        w_d[name] = nc.dram_tensor(name, shp, dt_, kind="ExternalInput").ap()
    out_d = nc.dram_tensor("out", [OUTP, S], f32, kind="ExternalOutput").ap()

    qctr = [0]

    with tile.TileContext(nc) as tc, nc.allow_low_precision("tol 2e-2"):
        with tc.tile_critical():
            nc.gpsimd.load_library(mlp)
        with (
            tc.tile_pool(name="const", bufs=1) as cp,
            tc.tile_pool(name="hT", bufs=2) as hTp,
            tc.tile_pool(name="msgs",
                         bufs=max(3, 24576 // (MAXT * P * 2))) as mp,
            tc.tile_pool(name="prep", bufs=max(NPREP, 1)) as ppool,
            tc.tile_pool(name="oh", bufs=5) as ohp,
            tc.tile_pool(name="sb", bufs=3) as sbp,
            tc.tile_pool(name="pagg", bufs=3, space="PSUM") as pagg,
            tc.tile_pool(name="ptr", bufs=2, space="PSUM") as ptr,
            tc.tile_pool(name="pd", bufs=2, space="PSUM") as pd,
            tc.tile_pool(name="dram", bufs=2, space="DRAM") as dp,
        ):
            def load_const(ap_, shp, dtype=f32, tag=None):
                t = cp.tile(shp, dtype, tag=tag or ap_.tensor.name)
                nc.sync.dma_start(out=t[:], in_=ap_)
                return t

            idx_sb = load_const(idx_d, [P, plan.icols], i16)
            dst_sb = load_const(dstloc_d, [P, plan.dcols], bf16)
            ddiag_sb = load_const(ddiag_d, [P, NB * P], bf16)
            iota_sb = load_const(w_d["iota"], [P, P], bf16)
            identb_sb = load_const(w_d["identb"], [P, P], bf16)
            ones_sb = load_const(w_d["ones"], [1, P])
            wsb = {}
            for k_ in ("wl0", "wr0", "wl1", "wr1", "wl2", "wr2"):
                wsb[k_] = load_const(w_d[k_], list(w_d[k_].shape), bf16)
            for k_ in ("bl0t", "bl1t", "bl2r"):
                wsb[k_] = load_const(w_d[k_], list(w_d[k_].shape), f32)

            hT_x = hTp.tile([P, S], bf16, tag="hT", name="hT_x")
            nc.sync.dma_start(out=hT_x[:], in_=xtloc)

            if EXACT:
                cnt_sb = load_const(percnt_d, [1, 2 * NB], mybir.dt.int32)
                cnt_reg = nc.alloc_register(mybir.EngineType.Pool, "gnncnt")

            PSEMS = ([nc.alloc_semaphore(f"gnnpq{q}") for q in range(NQ)]
                     if NPREP > 0 else None)
            PQC = [0] * NQ   # per-queue cumulative prep count

            iota_b = iota_sb[:].rearrange("p (a c) -> p a c", a=1)

            prep_state = {}

            def emit_preps(pass_id, part, source, nprep):
                """Generate descriptors for the first calls of a FUTURE pass
                (prepare_only): the Q7 does the gen work now, during idle
                time; the DMAs fire later via trigger_dma."""
                pb = plan.pbase[part]
                lst = []
                for k in range(min(nprep, len(plan.calls[part]))):
                    call = plan.calls[part][k]
                    t = ppool.tile([P, plan.maxt * HID], bf16, tag="prep")
                    q = qctr[0] % NQ
                    qctr[0] += 1
                    if EXACT:
                        # ring accounting must match the Q7's trailing-neg trim
                        nc.reg_load(cnt_reg,
                                    cnt_sb[0:1, part * NB + call.b0:
                                           part * NB + call.b0 + 1])
                        nreg = cnt_reg
                    else:
                        nreg = call.T * P
                    nc.gpsimd.dma_gather(
                        out_ap=t[:, :call.T * HID]
                            .rearrange("p (t c) -> p t c", c=HID),
                        in_ap=source,
                        idxs_ap=idx_sb[:, (pb + call.t0) * 8:
                                       (pb + call.t0 + call.T) * 8],
                        num_idxs=call.T * P,
                        num_idxs_reg=nreg,
                        elem_size=HID,
                        single_packet=sp_eff,
                        queue_num=q,
                        prepare_only=True,
                        sem=PSEMS[q],
                    )
                    PQC[q] += 1
                    lst.append((t, q))
                # consumers must wait for the queue's whole batch: per-engine
                # completion increments interleave across calls on a queue
                qtot = list(PQC)
                prep_state[pass_id] = [(t, q, qtot[q]) for (t, q) in lst]

            def emit_pass(part, source, is_stream, per_block, pass_id=None,
                          prefetch=None, blo=0, bhi=None):
                """Emit gathers/streams + one-hot builds + agg matmuls for
                blocks [blo, bhi) of `part` (group-aligned); call
                per_block(b, agg_ps) on each result."""
                pb = plan.pbase[part]
                calls = plan.calls[part]
                MAXT = plan.maxt
                mtiles = {}
                ci = [0]
                if bhi is None:
                    bhi = plan.nb
                if blo > 0:
                    # skip calls entirely before this block range (a call
                    # straddling the boundary is re-issued, which is harmless)
                    ts = plan.tcol[part][blo]
                    while (ci[0] < len(calls)
                           and calls[ci[0]].t0 + calls[ci[0]].T <= ts):
                        ci[0] += 1
                preps = prep_state.pop(pass_id, [])
                trig = set()

                def issue_thru(t_end):
                    while ci[0] < len(calls) and calls[ci[0]].t0 < t_end:
                        call = calls[ci[0]]
                        if ci[0] < len(preps):
                            t, q, qtot = preps[ci[0]]
                            if q not in trig:
                                nc.gpsimd.trigger_dma(count=None, queue_num=q)
                                trig.add(q)
                            # gate the consuming matmuls (PE) on the DMA
                            # completion sem — Tile only orders them after
                            # the prep's dispatch, not the data landing
                            nc.tensor.wait_ge(PSEMS[q], 16 * qtot)
                            mtiles[ci[0]] = t
                            ci[0] += 1
                            continue
                        msgs = mp.tile([P, MAXT * HID], bf16, tag="msgs")
                        mv = msgs[:, :call.T * HID]
                        if is_stream:
                            # (spreading these dma_starts across sync+scalar
                            # was measured SLOWER, 804us vs 767: the dispatch
                            # interferes with the Scalar engine's copy/relu)
                            nc.sync.dma_start(
                                out=mv,
                                in_=source[:, call.t0 * HID:
                                           (call.t0 + call.T) * HID])
                        else:
                            if EXACT:
                                cid = part * NB + call.b0
                                nc.reg_load(
                                    cnt_reg, cnt_sb[0:1, cid:cid + 1])
                                nreg = cnt_reg
                            else:
                                nreg = call.T * P
                            nc.gpsimd.dma_gather(
                                out_ap=mv.rearrange("p (t c) -> p t c", c=HID),
                                in_ap=source,
                                idxs_ap=idx_sb[:, (pb + call.t0) * 8:
                                               (pb + call.t0 + call.T) * 8],
                                num_idxs=call.T * P,
                                num_idxs_reg=nreg,
                                elem_size=HID,
                                single_packet=sp_eff,
                                queue_num=qctr[0] % NQ,
                            )
                            qctr[0] += 1
                        mtiles[ci[0]] = msgs
                        ci[0] += 1

                oh_dma = OHMODE == "1" or (OHMODE == "l0" and is_stream)
                for g in [g for g in plan.groups[part]
                          if blo <= g.b0 < bhi]:
                    if oh_dma:
                        O = ohp.tile([P, plan.oht * P], mybir.dt.float8e4,
                                     tag="oh")
                        nc.scalar.dma_start(
                            out=O[:, :g.T * P],
                            in_=ohs_d[:, (pb + g.t0) * P:
                                      (pb + g.t0 + g.T) * P])
                    else:
                        # fp8 output halves the DVE write bytes; iota as in0
                        # gives the contiguous inner stride.  (0/1 are exact
                        # in fp8e4; the PE takes mixed fp8xbf16 operands.)
                        O = ohp.tile([P, plan.oht * P], mybir.dt.float8e4,
                                     tag="oh")
                        nc.vector.tensor_tensor(
                            out=O[:, :g.T * P]
                                .rearrange("p (t c) -> p t c", t=g.T),
                            in0=iota_b.to_broadcast([P, g.T, P]),
                            in1=dst_sb[:, pb + g.t0:pb + g.t0 + g.T]
                                .to_broadcast([P, g.T, P]),
                            op=OP.is_equal,
                        )
                    for b in range(g.b0, g.b0 + g.nblocks):
                        tn = plan.tsub[b][part]
                        if tn == 0:
                            per_block(b, None)
                            continue
                        t0b = plan.tcol[part][b]
                        issue_thru(t0b + tn)
                        go = t0b - g.t0            # subtile offset in O
                        agg_ps = pagg.tile([P, HID], f32, tag="agg")
                        for t in range(tn):
                            tg = t0b + t
                            k, off = plan.call_of_t[part][tg]
                            nc.tensor.matmul(
                                out=agg_ps[:],
                                lhsT=O[:, (go + t) * P:(go + t + 1) * P],
                                rhs=mtiles[k][:, off * HID:(off + 1) * HID],
                                start=(t == 0), stop=(t == tn - 1),
                            )
                        per_block(b, agg_ps)
                if prefetch is not None:
                    emit_preps(*prefetch)

            def pass_a(part, source, is_stream, aggbuf, pass_id=None,
                       prefetch=None, blo=0, bhi=None):
                def pb_(b, agg_ps):
                    if agg_ps is None:
                        nc.vector.memset(aggbuf[:, b * P:(b + 1) * P], 0)
                    else:
                        nc.scalar.activation(aggbuf[:, b * P:(b + 1) * P],
                                             agg_ps[:], AF.Copy)
                emit_pass(part, source, is_stream, pb_, pass_id, prefetch,
                          blo, bhi)

            def pass_b(li, part, source, is_stream, aggbuf, wl, wr, bias,
                       hT_prev, hT_cur, bounce1, bounce2, pass_id=None,
                       prefetch=None, blo=0, bhi=None):
                K = OUTP if li == 2 else HID

                def pb_(b, agg_ps):
                    bs = slice(b * P, (b + 1) * P)
                    mean_pre = sbp.tile([P, HID], bf16, tag="mean_pre")
                    if agg_ps is not None:
                        nc.vector.tensor_tensor(
                            out=mean_pre[:], in0=aggbuf[:, bs],
                            in1=agg_ps[:], op=OP.add)
                    else:
                        nc.vector.tensor_copy(out=mean_pre[:], in_=aggbuf[:, bs])
                    # fused transpose+scale: meanT = mean_pre^T @ diag(dinv_b)
                    mt_ps = ptr.tile([P, P], f32, tag="mt")
                    nc.tensor.matmul(out=mt_ps[:], lhsT=mean_pre[:],
                                     rhs=ddiag_sb[:, bs], start=True, stop=True)
                    meanT = sbp.tile([P, P], bf16, tag="meanT")
                    nc.scalar.activation(meanT[:], mt_ps[:], AF.Copy)

                    dens = pd.tile([K, P], f32, tag="dens")
                    nc.tensor.matmul(out=dens[:], lhsT=wl[:, :K], rhs=meanT[:],
                                     start=True, stop=False)
                    if li == 2:
                        nc.tensor.matmul(out=dens[:], lhsT=wr[:, :K],
                                         rhs=hT_prev[:, bs],
                                         start=False, stop=False)
                        nc.tensor.matmul(out=dens[:], lhsT=bias[:],
                                         rhs=ones_sb[:], start=False, stop=True)
                        out_sb = sbp.tile([OUTP, P], f32, tag="out_sb")
                        nc.scalar.activation(out_sb[:], dens[:], AF.Copy)
                        nc.sync.dma_start(out=out_d[:, bs], in_=out_sb[:])
                    else:
                        nc.tensor.matmul(out=dens[:], lhsT=wr[:, :K],
                                         rhs=hT_prev[:, bs],
                                         start=False, stop=True)
                        nc.scalar.activation(hT_cur[:, bs], dens[:], AF.Relu,
                                             bias=bias[:])
                        h_ps = ptr.tile([P, P], bf16, tag="hps", bufs=1)
                        nc.tensor.transpose(h_ps[:], hT_cur[:, bs], identb_sb[:])
                        h_b = sbp.tile([P, HID], bf16, tag="h_b")
                        nc.scalar.activation(h_b[:], h_ps[:], AF.Copy)
                        if b < NB1:
                            nc.sync.dma_start(
                                out=bounce1[b * P:(b + 1) * P, :], in_=h_b[:])
                        else:
                            bb = b - NB1
                            nc.sync.dma_start(
                                out=bounce2[bb * P:(bb + 1) * P, :], in_=h_b[:])
                emit_pass(part, source, is_stream, pb_, pass_id, prefetch,
                          blo, bhi)

            def allgather(bounce, full):
                nc.gpsimd.collective_compute(
                    "AllGather", OP.bypass,
                    replica_groups=[list(range(cfg.n_cores))],
                    ins=[bounce.opt()], outs=[full.opt()],
                )

            # DRAM staging (double buffered across layers)
            tb1 = [dp.tile([cfg.p1_rows, HID], bf16, tag="t1",
                           addr_space="Shared", name=f"t1_{i}") for i in range(2)]
            tb2 = [dp.tile([cfg.p2_rows, HID], bf16, tag="t2",
                           addr_space="Shared", name=f"t2_{i}") for i in range(2)]
            bn1 = [dp.tile([cfg.nb1 * P, HID], bf16, tag="b1", name=f"b1_{i}")
                   for i in range(2)]
            bn2 = [dp.tile([cfg.nb2 * P, HID], bf16, tag="b2", name=f"b2_{i}")
                   for i in range(2)]

            hT = [hTp.tile([P, S], bf16, tag="hT", name=f"hT{i}")
                  for i in range(2)]
            ab = [hTp.tile([P, S], bf16, tag="aggbuf", name=f"ab{i}")
                  for i in range(2)]

            # Prefetch chain: each pass generates (prepare_only) the gather
            # descriptors for the FIRST NPREP calls of the NEXT gather pass
            # at the end of its own emission, so the Q7 gen work overlaps
            # the AllGather wait / compute tail. Fired via trigger_dma.
            def pf(pid, part, src):
                return (pid, part, src) + (NPREP,) if NPREP > 0 else None

            def prep_barrier(tbl):
                # The preps are emitted before the table's AllGather exists
                # in the IR, so no RAW dep reaches them or their triggers.
                # This gpsimd read of the table blocks the engine (and hence
                # the triggers behind it in FIFO order) until the data lands.
                if NPREP > 0:
                    bsc = sbp.tile([1, HID], bf16, tag="bar")
                    nc.sync.dma_start(out=bsc[:], in_=tbl[0:1, 0:HID])
                    bdst = sbp.tile([16, HID], bf16, tag="bar2")
                    nc.gpsimd.partition_broadcast(bdst[:], bsc[:], channels=16)

            # layer 0 (host-pregathered msgs, streamed via HWDGE)
            # (a two-half split to launch bounce1's AllGather earlier was
            # measured SLOWER — 1083 vs 1048 us — the collective dispatch
            # is dep-driven and already early; the split only disrupted
            # cross-pass pipelining)
            pass_a(0, l0m1_d, STREAM0, ab[0])
            pass_b(0, 1, l0m2_d, STREAM0, ab[0], wsb["wl0"], wsb["wr0"],
                   wsb["bl0t"], hT_x, hT[0], bn1[0][:], bn2[0][:],
                   prefetch=pf("l1p1", 0, tb1[0][:]))
            allgather(bn1[0], tb1[0])
            allgather(bn2[0], tb2[0])

            # layer 1
            prep_barrier(tb1[0])
            pass_a(0, tb1[0][:], False, ab[1], pass_id="l1p1",
                   prefetch=pf("l1p2", 1, tb2[0][:]))
            prep_barrier(tb2[0])
            pass_b(1, 1, tb2[0][:], False, ab[1], wsb["wl1"], wsb["wr1"],
                   wsb["bl1t"], hT[0], hT[1], bn1[1][:], bn2[1][:],
                   pass_id="l1p2", prefetch=pf("l2p1", 0, tb1[1][:]))
            allgather(bn1[1], tb1[1])
            allgather(bn2[1], tb2[1])

            # layer 2
            prep_barrier(tb1[1])
            pass_a(0, tb1[1][:], False, ab[0], pass_id="l2p1",
                   prefetch=pf("l2p2", 1, tb2[1][:]))
            prep_barrier(tb2[1])
            pass_b(2, 1, tb2[1][:], False, ab[0], wsb["wl2"], wsb["wr2"],
                   wsb["bl2r"], hT[1], None, None, None, pass_id="l2p2")

    nc.compile()
    return nc


# --------------------------------------------------------------------------
# Entry point
# --------------------------------------------------------------------------

def _make_in_maps(cfg: Cfg, host, weights):
    iota = np.broadcast_to(np.arange(P, dtype=np.float32), (P, P)).astype(BF16)
    ident = np.eye(P, dtype=np.float32)
    ones = np.ones((1, P), np.float32)
    maps = []
    for i in range(cfg.n_cores):
        m = dict(
            xtloc=host["xT"][i],
            idx=host["idx"][i],
            dstloc=host["dstloc"][i],
            ddiag=host["ddiag"][i],
            iota=iota, identb=ident.astype(BF16), ones=ones,
        )
        if OHSTREAM:
            m["ohs"] = host["ohs"][i]
        if EXACT:
            m["percnt"] = host["percnt"][i]
        if STREAM0:
            m["l0m1"] = host["l0m1"][i]
            m["l0m2"] = host["l0m2"][i]
        else:
            m["xp1"] = host["xp1"]
            m["xp2"] = host["xp2"]
        m.update(weights)
        maps.append(m)
    return maps


def _pad_w(w, outp):
    w = np.asarray(w, np.float32)
    if w.shape[-1] < outp:
        pad = np.zeros(w.shape[:-1] + (outp - w.shape[-1],), np.float32)
        w = np.concatenate([w, pad], axis=-1)
    return w


def _ensure_ntff_hook():
    import types

    try:
        from antenv.axon_hooks import get_axon_ntff_profile_hook  # noqa: F401
        return
    except ImportError:
        pass
    try:
        import antenv
        from trn_agent_boot.trn_boot import _ntff_profile_via_ctypes
    except ImportError:
        return
    hook = _ntff_profile_via_ctypes("/opt/axon/libaxon_pjrt.so")
    mod = types.ModuleType("antenv.axon_hooks")
    mod.get_axon_ntff_profile_hook = lambda: hook
    mod.set_axon_ntff_profile_hook = lambda h: None
    sys.modules["antenv.axon_hooks"] = mod
    antenv.axon_hooks = mod


def run(cfg: Cfg, inputs, trace=False):
    import concourse.bass_utils as bu
    from concourse.bass_utils import run_bass_kernel_spmd

    if trace:
        _ensure_ntff_hook()
        bu.upload_artifacts = lambda d: str(d)

    x = np.asarray(inputs["x"], np.float32)
    ei = np.asarray(inputs["edge_index"])
    src = ei[0].astype(np.int64)
    dst = ei[1].astype(np.int64)

    host = prep_host(cfg, x, src, dst)
    weights = dict(
        wl0=np.asarray(inputs["Wl0"], np.float32).astype(BF16),
        wr0=np.asarray(inputs["Wr0"], np.float32).astype(BF16),
        wl1=np.asarray(inputs["Wl1"], np.float32).astype(BF16),
        wr1=np.asarray(inputs["Wr1"], np.float32).astype(BF16),
        wl2=_pad_w(inputs["Wl2"], cfg.out_p).astype(BF16),
        wr2=_pad_w(inputs["Wr2"], cfg.out_p).astype(BF16),
        bl0t=np.asarray(inputs["bl0"], np.float32).reshape(-1, 1),
        bl1t=np.asarray(inputs["bl1"], np.float32).reshape(-1, 1),
        bl2r=_pad_w(np.asarray(inputs["bl2"], np.float32).reshape(1, -1),
                    cfg.out_p),
    )

    nc = build_program(cfg, host["plan"])
    in_maps = _make_in_maps(cfg, host, weights)
    res = run_bass_kernel_spmd(nc, in_maps, core_ids=list(range(cfg.n_cores)),
                               trace=trace)

    out_full = np.empty((cfg.n_nodes, cfg.out_f), np.float32)
    node_of_pos = host["node_of_pos"]
    for i in range(cfg.n_cores):
        o = np.asarray(res.results[i]["out"]).T    # [S, OUTP]
        pos = np.arange(i * cfg.slots_core, (i + 1) * cfg.slots_core)
        nodes = node_of_pos[pos]
        valid = nodes >= 0
        out_full[nodes[valid]] = o[valid][:, :cfg.out_f]
    return out_full, res


def kernel(**inputs) -> np.ndarray:
    trace = os.environ.get("GNN_TRACE", "0") == "1"
    out, _ = run(FULL_CFG, inputs, trace=trace)
    return out

